# revision 1
# baseline (speedup 1.0000x reference)
"""Trainium2 Bass kernel for nn_ChunkData (sliding-window chunk gather).

Reference computation (T=16384, F=257, C=64, N=T-C=16320):
    x[i, f, j] = mixed_mag[i + j, f]   # [N, F, C]
    y[i, f]    = clean_mag[i + C, f]   # [N, F]

Pure data movement; output x is ~1.07 GB so the kernel is HBM-write-bandwidth
bound. Strategy (per the data-parallel sharding hint):

  - Shard the chunk dimension N across 8 NeuronCores (2040 chunks each).
    Core k gets rows [2040k, 2040k + 2104) of mixed_mag, pre-transposed on
    host to f-major [257, 2104] so that each chunk's inner window is
    contiguous in the SBUF free dimension.
  - On device, frequency rows are loaded into SBUF with TWO adjacent f-rows
    packed per partition (even rows -> one tile, odd rows -> another). A DVE
    (vector engine) pass stages each block of Bo=136 chunks into an SBUF
    tile whose per-partition layout is, for each chunk i:
        [row_{2p}[i:i+64] ++ row_{2p+1}[i:i+64]]   (512 B contiguous)
    This makes every DMA descriptor 512 B instead of 256 B, which avoids the
    sub-512B read-modify-write penalty in the SDMA engines (measured 1.35x).
  - The staged tile is DMA'd straight into the final [N, F, C] layout
    (per chunk, f-pair p writes x[i, 2p:2p+2, :] = 512 B contiguous).
    The single f=256 row is written by a separate small windowed DMA.
  - y is a contiguous DRAM->DRAM copy of the pre-sliced clean_mag rows.

Measured ~475-507 us per core for the 134 MB shard (~270-283 GB/s), which
matches the per-NC DMA write-bandwidth ceiling measured with pure contiguous
64 KB-descriptor writes (~267 GB/s) - i.e. the kernel is at the roofline.
"""

import numpy as np

import concourse.bass as bass
import concourse.bacc as bacc
import concourse.mybir as mybir
from concourse.tile import TileContext
from concourse.bass_utils import run_bass_kernel_spmd

N_CORES = 8
T_FULL, F, C = 16384, 257, 64
N_FULL = T_FULL - C            # 16320 output chunks
N_PER = N_FULL // N_CORES      # 2040 chunks per core
T_PER = N_PER + C              # 2104 input rows per core
FC = F * C                     # 16448 elems per output chunk
Bo = 136                       # chunks per staged block (2040 = 15 * 136)
SROW = Bo * 2 * C              # stage elems per partition per block


def _build(loop_k=0, timing=False):
    """Build the SPMD Bass program.

    loop_k=0: single pass, real ExternalOutputs (production).
    loop_k>0 with timing=True: store phase wrapped in a For_i(loop_k) loop
    with x_out kept in device DRAM (Internal) so wall-clock deltas between
    two loop_k values isolate pure on-device execution time.
    """
    nc = bacc.Bacc(None, target_bir_lowering=False)
    mmT = nc.dram_tensor("mmT", [F, T_PER], mybir.dt.float32, kind="ExternalInput")
    cl = nc.dram_tensor("cl", [N_PER, F], mybir.dt.float32, kind="ExternalInput")
    xkind = "Internal" if timing else "ExternalOutput"
    x_out = nc.dram_tensor("x_out", [N_PER, F, C], mybir.dt.float32, kind=xkind)
    y_out = nc.dram_tensor("y_out", [N_PER, F], mybir.dt.float32,
                           kind="Internal" if timing else "ExternalOutput")
    outtok = nc.dram_tensor("outtok", [1, 16], mybir.dt.float32,
                            kind="ExternalOutput")

    with TileContext(nc) as tc:
        with tc.tile_pool(name="inp", bufs=1) as ipool, \
             tc.tile_pool(name="stg", bufs=2) as spool:
            t_even = ipool.tile([128, T_PER], mybir.dt.float32, name="t_even")
            t_odd = ipool.tile([128, T_PER], mybir.dt.float32, name="t_odd")
            t_last = ipool.tile([128, T_PER], mybir.dt.float32, name="t_last")
            # f-major loads: partition p holds rows 2p / 2p+1 / 256.
            nc.sync.dma_start(t_even[:, :],
                              bass.AP(mmT, 0, [[2 * T_PER, 128], [1, T_PER]]))
            nc.sync.dma_start(t_odd[:, :],
                              bass.AP(mmT, T_PER, [[2 * T_PER, 128], [1, T_PER]]))
            nc.sync.dma_start(t_last[0:1, :],
                              bass.AP(mmT, 256 * T_PER, [[T_PER, 1], [1, T_PER]]))
            nc.scalar.dma_start(y_out[:, :], cl[:, :])

            def store_phase():
                for b in range(N_PER // Bo):
                    i0 = b * Bo
                    stage = spool.tile([128, SROW], mybir.dt.float32, name="stage",
                                       tag="stage")
                    # Pack: stage[p, i*128 + 0:64]   = row_{2p}[i0+i : i0+i+64]
                    #       stage[p, i*128 + 64:128] = row_{2p+1}[...]
                    nc.vector.tensor_copy(
                        bass.AP(stage.tensor, 0, [[SROW, 128], [2 * C, Bo], [1, C]]),
                        bass.AP(t_even.tensor, i0, [[T_PER, 128], [1, Bo], [1, C]]))
                    nc.vector.tensor_copy(
                        bass.AP(stage.tensor, C, [[SROW, 128], [2 * C, Bo], [1, C]]),
                        bass.AP(t_odd.tensor, i0, [[T_PER, 128], [1, Bo], [1, C]]))
                    # x[i, 0:256, :] for Bo chunks in one DMA, 512-B descriptors.
                    nc.sync.dma_start(
                        bass.AP(x_out, i0 * FC, [[2 * C, 128], [FC, Bo], [1, 2 * C]]),
                        bass.AP(stage.tensor, 0, [[SROW, 128], [2 * C, Bo], [1, 2 * C]]))
                    # x[i, 256, :] tail row, windowed directly from t_last.
                    nc.scalar.dma_start(
                        bass.AP(x_out, i0 * FC + 256 * C, [[C, 1], [FC, Bo], [1, C]]),
                        bass.AP(t_last.tensor, i0, [[T_PER, 1], [1, Bo], [1, C]]))

            if loop_k:
                with tc.For_i(0, loop_k):
                    store_phase()
            else:
                store_phase()
            nc.sync.dma_start(outtok[:, :], t_even[0:1, 0:16])
    nc.compile()
    return nc


def _in_maps(mixed_mag, clean_mag):
    mixed_mag = np.ascontiguousarray(np.asarray(mixed_mag), dtype=np.float32)
    clean_mag = np.ascontiguousarray(np.asarray(clean_mag), dtype=np.float32)
    assert mixed_mag.shape == (T_FULL, F), mixed_mag.shape
    assert clean_mag.shape == (T_FULL, F), clean_mag.shape
    maps = []
    for k in range(N_CORES):
        r0 = k * N_PER
        maps.append({
            "mmT": np.ascontiguousarray(mixed_mag[r0:r0 + T_PER].T),
            "cl": np.ascontiguousarray(clean_mag[r0 + C:r0 + C + N_PER]),
        })
    return maps


def kernel(mixed_mag, clean_mag):
    in_maps = _in_maps(mixed_mag, clean_mag)
    nc = _build()
    res = run_bass_kernel_spmd(nc, in_maps, core_ids=list(range(N_CORES)))
    x = np.concatenate([r["x_out"] for r in res.results], axis=0)
    y = np.concatenate([r["y_out"] for r in res.results], axis=0)
    return x, y


# revision 3
# speedup vs baseline: 1.0451x; 1.0451x over previous
"""Trainium2 Bass kernel for nn_ChunkData (sliding-window chunk gather).

Reference computation (T=16384, F=257, C=64, N=T-C=16320):
    x[i, f, j] = mixed_mag[i + j, f]   # [N, F, C]
    y[i, f]    = clean_mag[i + C, f]   # [N, F]

Pure data movement; output x is ~1.07 GB so the kernel is HBM-write-bandwidth
bound. Strategy (per the data-parallel sharding hint):

  - Shard the chunk dimension N across 8 NeuronCores (2040 chunks each).
    Core k gets rows [2040k, 2040k + 2104) of mixed_mag, pre-transposed on
    host to f-major [257, 2104] so each chunk's window is contiguous in the
    SBUF free dimension.
  - On device, R=4 adjacent frequency rows are packed per SBUF partition.
    The 128 partitions are split into G=2 groups of 64; group g covers
    chunk sub-range [g*1020, (g+1)*1020). Input tiles t_r (r = f mod 4) are
    loaded so partition p holds row 4*(p%64)+r time-shifted by 1020*(p//64),
    which lets every DVE staging op span all 128 partitions with one AP.
  - A DVE (vector engine) pass stages each block of Bo=60 chunks (per
    group; 120 chunks per stage tile) into a layout where, per partition
    and chunk, the 4 packed rows' windows are contiguous:
        [row_{4q}[i:i+64] ... row_{4q+3}[i:i+64]]   (1024 B)
    so every output DMA descriptor is 1024 B. This avoids the sub-512B
    SDMA read-modify-write penalty (256-B direct descriptors measure ~1.5x
    slower) and halves per-descriptor overhead vs 512-B packing (~13%).
  - Each stage tile is written by two DMAs (one per partition group, on the
    two HWDGE queues so their disjoint SDMA engine halves run concurrently)
    straight into the final [N, F, C] layout. The single f=256 row is
    written by small windowed DMAs on the SWDGE (gpsimd) queue.
  - y is a contiguous DRAM->DRAM copy of the pre-sliced clean_mag rows.

Measured ~450 us per core for the 134 MB x-shard (~300 GB/s effective),
at the per-NC DMA write-bandwidth ceiling (pure contiguous 64KB-descriptor
writes measure ~267-290 GB/s on this part) - i.e. at the memory roofline.
"""

import numpy as np

import concourse.bass as bass
import concourse.bacc as bacc
import concourse.mybir as mybir
from concourse.tile import TileContext
from concourse.bass_utils import run_bass_kernel_spmd

N_CORES = 8
T_FULL, F, C = 16384, 257, 64
N_FULL = T_FULL - C            # 16320 output chunks
N_PER = N_FULL // N_CORES      # 2040 chunks per core
T_PER = N_PER + C              # 2104 input rows per core
FC = F * C                     # 16448 elems per output chunk

R = 4                          # f-rows packed per partition
P_g = 256 // R                 # 64 partitions per group
G = 128 // P_g                 # 2 chunk-range groups per stage tile
NG = N_PER // G                # 1020 chunks per group range
W = NG + C                     # 1084: input tile width per partition
Bo = 60                        # chunks per staged block per group (1020 = 17*60)
SROW = Bo * C * R              # stage elems per partition per block


def _build(loop_k=0, timing=False):
    """Build the SPMD Bass program.

    loop_k=0: single pass, real ExternalOutputs (production).
    loop_k>0 with timing=True: store phase wrapped in a For_i(loop_k) loop
    with x_out kept in device DRAM (Internal) so wall-clock deltas between
    two loop_k values isolate pure on-device execution time.
    """
    nc = bacc.Bacc(None, target_bir_lowering=False)
    mmT = nc.dram_tensor("mmT", [F, T_PER], mybir.dt.float32, kind="ExternalInput")
    cl = nc.dram_tensor("cl", [N_PER, F], mybir.dt.float32, kind="ExternalInput")
    xkind = "Internal" if timing else "ExternalOutput"
    x_out = nc.dram_tensor("x_out", [N_PER, F, C], mybir.dt.float32, kind=xkind)
    y_out = nc.dram_tensor("y_out", [N_PER, F], mybir.dt.float32,
                           kind="Internal" if timing else "ExternalOutput")
    outtok = nc.dram_tensor("outtok", [1, 16], mybir.dt.float32,
                            kind="ExternalOutput")

    with TileContext(nc) as tc:
        with tc.tile_pool(name="inp", bufs=1) as ipool, \
             tc.tile_pool(name="stg", bufs=2) as spool:
            t_in = [ipool.tile([128, W], mybir.dt.float32, name=f"t_in{r}",
                               tag=f"t{r}") for r in range(R)]
            t_last = ipool.tile([128, T_PER], mybir.dt.float32, name="t_last",
                                tag="tl")
            # t_in[r][p, t] = mmT[R*(p%P_g) + r, t + NG*(p//P_g)]
            for r in range(R):
                nc.sync.dma_start(
                    t_in[r][:, :],
                    bass.AP(mmT, r * T_PER, [[NG, G], [R * T_PER, P_g], [1, W]]))
            nc.sync.dma_start(t_last[0:1, :],
                              bass.AP(mmT, 256 * T_PER, [[T_PER, 1], [1, T_PER]]))
            nc.scalar.dma_start(y_out[:, :], cl[:, :])

            def store_phase():
                for b in range(NG // Bo):
                    i0 = b * Bo
                    stage = spool.tile([128, SROW], mybir.dt.float32,
                                       name="stage", tag="stage")
                    # stage[p, i*R*C + r*C + j] = t_in[r][p, i0 + i + j]
                    for r in range(R):
                        nc.vector.tensor_copy(
                            bass.AP(stage.tensor, r * C,
                                    [[SROW, 128], [R * C, Bo], [1, C]]),
                            bass.AP(t_in[r].tensor, i0,
                                    [[W, 128], [1, Bo], [1, C]]))
                    # One DMA per partition group; disjoint SDMA engine halves
                    # run concurrently off the two HWDGE rings.
                    for g in range(G):
                        q = nc.sync if g < max(1, G // 2) else nc.scalar
                        q.dma_start(
                            bass.AP(x_out, (i0 + g * NG) * FC,
                                    [[R * C, P_g], [FC, Bo], [1, R * C]]),
                            bass.AP(stage.tensor, g * P_g * SROW,
                                    [[SROW, P_g], [R * C, Bo], [1, R * C]]))
                    # x[i, 256, :] tail row, windowed from t_last.
                    for g in range(G):
                        nc.scalar.dma_start(
                            bass.AP(x_out, (i0 + g * NG) * FC + 256 * C,
                                    [[C, 1], [FC, Bo], [1, C]]),
                            bass.AP(t_last.tensor, i0 + g * NG,
                                    [[T_PER, 1], [1, Bo], [1, C]]))

            if loop_k:
                with tc.For_i(0, loop_k):
                    store_phase()
            else:
                store_phase()
            nc.sync.dma_start(outtok[:, :], t_in[0][0:1, 0:16])
    nc.compile()
    return nc


def _in_maps(mixed_mag, clean_mag):
    mixed_mag = np.ascontiguousarray(np.asarray(mixed_mag), dtype=np.float32)
    clean_mag = np.ascontiguousarray(np.asarray(clean_mag), dtype=np.float32)
    assert mixed_mag.shape == (T_FULL, F), mixed_mag.shape
    assert clean_mag.shape == (T_FULL, F), clean_mag.shape
    maps = []
    for k in range(N_CORES):
        r0 = k * N_PER
        maps.append({
            "mmT": np.ascontiguousarray(mixed_mag[r0:r0 + T_PER].T),
            "cl": np.ascontiguousarray(clean_mag[r0 + C:r0 + C + N_PER]),
        })
    return maps


def kernel(mixed_mag, clean_mag):
    in_maps = _in_maps(mixed_mag, clean_mag)
    nc = _build()
    res = run_bass_kernel_spmd(nc, in_maps, core_ids=list(range(N_CORES)))
    x = np.concatenate([r["x_out"] for r in res.results], axis=0)
    y = np.concatenate([r["y_out"] for r in res.results], axis=0)
    return x, y
